# revision 5
# baseline (speedup 1.0000x reference)
"""Trainium2 Bass kernel for DynamicResidualStageWrapper (18-block MLP stage
with channel-gated anchor routing), data-parallel over batch across 8 cores.

Contract: kernel(**inputs) takes FULL unsharded inputs (as numpy arrays, keyed
as in reference.setup_inputs()) and returns the FULL output [32,14,14,512].
"""

import os
import numpy as np

import concourse.bacc as bacc
import concourse.bass as bass
import concourse.mybir as mybir
import concourse.tile as tile
from concourse.bass_utils import run_bass_kernel_spmd

# ---- problem constants (hardcoded per spec) ----
NUM_BLOCKS = 18
ANCHOR_IDX = (1, 4, 9)
TARGET_IDX = (11, 14, 17)
C = 512
HID = 128
A = 3
B, H, W = 32, 14, 14
N_CORES = 8
BL = B // N_CORES          # 4 samples per core
HW = H * W                 # 196 positions per sample
T = BL * HW                # 784 tokens per core
KT = C // 128              # 4 channel tiles
NCH = 2                    # token chunks per core
CH = T // NCH              # 392 tokens per chunk
SPC = BL // NCH            # 2 samples per chunk

F32 = mybir.dt.float32
# float32r: full-rate fp32 matmul mode on the PE (needs moving dim >= 256).
# All matmul operands (tiles and the DRAM tensors feeding them) are typed
# float32r; the host pre-rounds them to the representable set (bf16 hi+lo).
USE_F32R = os.environ.get("KBENCH_MM_DT", "f32r") == "f32r"
MM = mybir.dt.float32r if USE_F32R else F32
GELU = mybir.ActivationFunctionType.Gelu_apprx_tanh
TANH = mybir.ActivationFunctionType.Tanh

_cached = {}


def build_program():
    """Build the per-core Bass/Tile program (same program on all 8 cores)."""
    nc = bacc.Bacc(trn_type="TRN2", target_bir_lowering=False, debug=False)

    xT = nc.dram_tensor("xT", [C, T], MM, kind="ExternalInput").ap()
    wd = nc.dram_tensor("wd", [NUM_BLOCKS, C, C], MM, kind="ExternalInput").ap()
    bias_cols = nc.dram_tensor("bias_cols", [128, NUM_BLOCKS * KT], F32,
                               kind="ExternalInput").ap()
    fc1w = nc.dram_tensor("fc1w", [128, A * KT * 128], MM, kind="ExternalInput").ap()
    fc1b = nc.dram_tensor("fc1b", [128, A], F32, kind="ExternalInput").ap()
    fc2w = nc.dram_tensor("fc2w", [128, A * A * C], MM, kind="ExternalInput").ap()
    fc2bias = nc.dram_tensor("fc2bias", [128, A * A * KT * BL], F32,
                             kind="ExternalInput").ap()
    gbc = nc.dram_tensor("gbc", [128, A], F32, kind="ExternalInput").ap()
    outT = nc.dram_tensor("outT", [C, T], MM, kind="ExternalOutput").ap()

    anchor_of = {b: i for i, b in enumerate(ANCHOR_IDX)}
    target_of = {b: i for i, b in enumerate(TARGET_IDX)}

    with tile.TileContext(nc) as tc:
        with (
            tc.tile_pool(name="const", bufs=1) as cpool,
            tc.tile_pool(name="wpool", bufs=3) as wpool,
            tc.tile_pool(name="xpool", bufs=3) as xpool,
            tc.tile_pool(name="apool", bufs=1) as apool,
            tc.tile_pool(name="rpool", bufs=2) as rpool,
            tc.tile_pool(name="ppool", bufs=6, space="PSUM") as ppool,
            tc.tile_pool(name="fcps", bufs=1, space="PSUM") as fcps,
        ):
            # ---- resident constants ----
            bias_t = cpool.tile([128, NUM_BLOCKS * KT], F32, name="bias_t")
            nc.sync.dma_start(bias_t[:], bias_cols[:])
            fc1w_t = cpool.tile([128, A * KT * 128], MM, name="fc1w_t")
            nc.sync.dma_start(fc1w_t[:], fc1w[:])
            fc1b_t = cpool.tile([128, A], F32, name="fc1b_t")
            nc.sync.dma_start(fc1b_t[:], fc1b[:])
            fc2w_t = cpool.tile([128, A * A * C], MM, name="fc2w_t")
            nc.sync.dma_start(fc2w_t[:], fc2w[:])
            fc2bias_t = cpool.tile([128, A * A * KT * BL], F32, name="fc2bias_t")
            nc.sync.dma_start(fc2bias_t[:], fc2bias[:])
            gbc_t = cpool.tile([128, A], F32, name="gbc_t")
            nc.sync.dma_start(gbc_t[:], gbc[:])

            # ---- load input activations ----
            X = {}
            for k in range(KT):
                for c in range(NCH):
                    xt = xpool.tile([128, CH], MM, tag=f"x{k}_{c}", name=f"xin{k}_{c}")
                    nc.sync.dma_start(
                        xt[:], xT[k * 128:(k + 1) * 128, c * CH:(c + 1) * CH])
                    X[k, c] = xt

            anchors = {}   # a -> {(k, c) -> tile}

            for i in range(NUM_BLOCKS):
                t_idx = target_of.get(i)
                a_idx = anchor_of.get(i)

                # stream this block's weights: 4 tiles [128 cin, 512 cout]
                wt = []
                for k in range(KT):
                    w_t = wpool.tile([128, C], MM, tag=f"w{k}", name=f"w{i}_{k}")
                    nc.sync.dma_start(w_t[:], wd[i, k * 128:(k + 1) * 128, :])
                    wt.append(w_t)

                pooled = None
                if t_idx is not None:
                    pooled = [rpool.tile([128, BL], F32, tag=f"pool{k}",
                                         name=f"pool{i}_{k}") for k in range(KT)]

                Xn = {}
                for c in range(NCH):
                    for ct in range(KT):
                        ps = ppool.tile([128, CH], F32, tag="mm",
                                        name=f"ps{i}_{ct}_{c}")
                        for k in range(KT):
                            nc.tensor.matmul(
                                ps[:],
                                wt[k][:, ct * 128:(ct + 1) * 128],
                                X[k, c][:],
                                start=(k == 0), stop=(k == KT - 1))
                        if a_idx is not None:
                            xn = apool.tile([128, CH], MM,
                                            tag=f"a{a_idx}_{ct}_{c}",
                                            name=f"anc{a_idx}_{ct}_{c}")
                        else:
                            xn = xpool.tile([128, CH], MM, tag=f"x{ct}_{c}",
                                            name=f"xb{i}_{ct}_{c}")
                        bias_ap = bias_t[:, i * KT + ct:i * KT + ct + 1]
                        if t_idx is None:
                            nc.scalar.activation(xn[:], ps[:], GELU, bias=bias_ap)
                        else:
                            # per-sample gelu with running sum -> pooled column
                            for s in range(SPC):
                                sl = slice(s * HW, (s + 1) * HW)
                                col = c * SPC + s
                                nc.scalar.activation(
                                    xn[:, sl], ps[:, sl], GELU, bias=bias_ap,
                                    accum_out=pooled[ct][:, col:col + 1])
                        Xn[ct, c] = xn

                if a_idx is not None:
                    anchors[a_idx] = Xn

                if t_idx is not None:
                    Xn = _routing(nc, rpool, xpool, fcps, t_idx, i, Xn, pooled,
                                  anchors, fc1w_t, fc1b_t, fc2w_t, fc2bias_t, gbc_t)
                X = Xn

            for k in range(KT):
                for c in range(NCH):
                    nc.sync.dma_start(
                        outT[k * 128:(k + 1) * 128, c * CH:(c + 1) * CH], X[k, c][:])

    nc.compile()
    return nc


def _routing(nc, rpool, xpool, fcps, t, blk, Xn, pooled, anchors,
             fc1w_t, fc1b_t, fc2w_t, fc2bias_t, gbc_t):
    """ChannelGating router: MLP on pooled features -> softmax over anchors ->
    weighted anchor sum added to Xn. Returns the updated activation tiles."""
    mul = mybir.AluOpType.mult
    add = mybir.AluOpType.add

    # round pooled (fp32 accumulators) to f32r for the fc1 matmul
    pooled_r = []
    for k in range(KT):
        pr = rpool.tile([128, BL], MM, tag=f"poolr{k}", name=f"poolr{t}_{k}")
        nc.vector.tensor_copy(pr[:], pooled[k][:])
        pooled_r.append(pr)
    # fc1: h = gelu(pooled @ fc1_w + fc1_b)   [HID=128, BL]
    ps1 = fcps.tile([128, BL], F32, tag="fcps", name=f"ps1_{t}")
    for k in range(KT):
        off = (t * KT + k) * 128
        nc.tensor.matmul(ps1[:], fc1w_t[:, off:off + 128], pooled_r[k][:],
                         start=(k == 0), stop=(k == KT - 1))
    h = rpool.tile([128, BL], MM, tag="h", name=f"h_{t}")
    nc.scalar.activation(h[:], ps1[:], GELU, bias=fc1b_t[:, t:t + 1])

    # fc2: logits [A*C, BL] as 12 col-tiles of one [128, 48] psum
    NJ = A * KT  # 12
    ps2 = fcps.tile([128, NJ * BL], F32, tag="fcps", name=f"ps2_{t}")
    for j in range(NJ):
        off = t * (A * C) + j * 128
        nc.tensor.matmul(ps2[:, j * BL:(j + 1) * BL],
                         fc2w_t[:, off:off + 128],
                         h[:], start=True, stop=True)
    logits = rpool.tile([128, NJ * BL], F32, tag="logits", name=f"lg_{t}")
    nc.vector.tensor_add(logits[:], ps2[:],
                         fc2bias_t[:, t * NJ * BL:(t + 1) * NJ * BL])

    # softmax over a (cols = a*16 + k*4 + b), exp via tanh identity:
    # e^x = (1 + tanh(x/2)) / (1 - tanh(x/2))
    KB = KT * BL  # 16
    lv = logits[:].rearrange("p (a kb) -> p kb a", a=A)
    m = rpool.tile([128, KB], F32, tag="m", name=f"m_{t}")
    nc.vector.tensor_reduce(m[:], lv, axis=mybir.AxisListType.X,
                            op=mybir.AluOpType.max)
    d = rpool.tile([128, A * KB], F32, tag="d", name=f"d_{t}")
    for a in range(A):
        nc.vector.tensor_sub(d[:, a * KB:(a + 1) * KB],
                             logits[:, a * KB:(a + 1) * KB], m[:])
    th = rpool.tile([128, A * KB], F32, tag="th", name=f"th_{t}")
    nc.scalar.activation(th[:], d[:], TANH, scale=0.5)
    num = rpool.tile([128, A * KB], F32, tag="num", name=f"num_{t}")
    nc.vector.tensor_scalar_add(num[:], th[:], 1.0)
    den = rpool.tile([128, A * KB], F32, tag="den", name=f"den_{t}")
    nc.vector.tensor_scalar(den[:], th[:], -1.0, 1.0, op0=mul, op1=add)
    rec = rpool.tile([128, A * KB], F32, tag="rec", name=f"rec_{t}")
    nc.vector.reciprocal(rec[:], den[:])
    e = rpool.tile([128, A * KB], F32, tag="e", name=f"e_{t}")
    nc.vector.tensor_mul(e[:], num[:], rec[:])
    s = rpool.tile([128, KB], F32, tag="s", name=f"s_{t}")
    nc.vector.tensor_reduce(s[:], e[:].rearrange("p (a kb) -> p kb a", a=A),
                            axis=mybir.AxisListType.X, op=add)
    rinv = rpool.tile([128, KB], F32, tag="rinv", name=f"rinv_{t}")
    nc.vector.reciprocal(rinv[:], s[:])
    rg = rpool.tile([128, KB], F32, tag="rg", name=f"rg_{t}")
    nc.vector.tensor_scalar_mul(rg[:], rinv[:], gbc_t[:, t:t + 1])
    g = rpool.tile([128, A * KB], F32, tag="g", name=f"g_{t}")
    for a in range(A):
        nc.vector.tensor_mul(g[:, a * KB:(a + 1) * KB],
                             e[:, a * KB:(a + 1) * KB], rg[:])

    # weighted anchor sum: xr = Xn + sum_a g_a * anchor_a  (per sample)
    Xr = {}
    for c in range(NCH):
        for k in range(KT):
            xr = xpool.tile([128, CH], MM, tag=f"x{k}_{c}", name=f"xr{t}_{k}_{c}")
            for s_ in range(SPC):
                b = c * SPC + s_
                sl = slice(s_ * HW, (s_ + 1) * HW)
                for a in range(A):
                    col = a * KB + k * BL + b
                    src = Xn[k, c] if a == 0 else xr
                    nc.vector.scalar_tensor_tensor(
                        xr[:, sl], anchors[a][k, c][:, sl],
                        g[:, col:col + 1], src[:, sl], op0=mul, op1=add)
            Xr[k, c] = xr
    return Xr


def _round_f32r(a):
    """Round fp32 to the f32r-representable set (bf16 hi + bf16 lo)."""
    if not USE_F32R:
        return np.ascontiguousarray(a, dtype=np.float32)
    import ml_dtypes
    a = np.asarray(a, dtype=np.float32)
    hi = a.astype(ml_dtypes.bfloat16).astype(np.float32)
    lo = (a - hi).astype(ml_dtypes.bfloat16).astype(np.float32)
    return np.ascontiguousarray(hi + lo)


def _prep_shared(block_w, block_b, fc1_w, fc1_b, fc2_w, fc2_b, gammas):
    """Host-side packing of the (replicated) weight tensors."""
    f = np.float32
    wd = np.ascontiguousarray(block_w, dtype=f)
    # bias column (i*KT+ct) = block_b[i, ct*128:(ct+1)*128]
    bias_cols = np.ascontiguousarray(
        block_b.reshape(NUM_BLOCKS * KT, 128).T, dtype=f)
    # fc1 with the mean-pool divisor folded in; col block (t*KT+k)
    fc1s = (fc1_w / float(HW)).astype(f)                      # [A, C, HID]
    fc1w_cat = np.concatenate(
        [fc1s[t][k * 128:(k + 1) * 128, :] for t in range(A) for k in range(KT)],
        axis=1)                                               # [128, A*KT*128]
    fc1b_cols = np.ascontiguousarray(np.asarray(fc1_b, dtype=f).T)  # [128, A]
    fc2w_cat = np.concatenate([np.asarray(fc2_w[t], dtype=f) for t in range(A)],
                              axis=1)                          # [128, A*A*C]
    # fc2 bias expanded to the [128, (a,k,b)] logits layout, repeated per b
    fc2bias = np.concatenate(
        [np.repeat(np.asarray(fc2_b[t], dtype=f).reshape(A * KT, 128).T,
                   BL, axis=1) for t in range(A)], axis=1)     # [128, A*A*KT*BL]
    gbc = np.broadcast_to(np.asarray(gammas, dtype=f)[None, :], (128, A))
    gbc = np.ascontiguousarray(gbc)
    return dict(wd=_round_f32r(wd), bias_cols=np.ascontiguousarray(bias_cols),
                fc1w=_round_f32r(fc1w_cat), fc1b=fc1b_cols,
                fc2w=_round_f32r(fc2w_cat),
                fc2bias=np.ascontiguousarray(fc2bias), gbc=gbc)


def shard_x(x):
    """Full x [B,H,W,C] -> per-core transposed shards [C, T]."""
    shards = []
    for r in range(N_CORES):
        xs = np.asarray(x[r * BL:(r + 1) * BL], dtype=np.float32)  # [BL,H,W,C]
        shards.append(_round_f32r(xs.reshape(T, C).T))             # [C, T]
    return shards


def unshard_out(outs):
    """Per-core [C, T] results -> full [B,H,W,C]."""
    parts = [o.T.reshape(BL, H, W, C) for o in outs]
    return np.ascontiguousarray(np.concatenate(parts, axis=0), dtype=np.float32)


def kernel(x, block_w, block_b, fc1_w, fc1_b, fc2_w, fc2_b, gammas):
    if "nc" not in _cached:
        _cached["nc"] = build_program()
    nc = _cached["nc"]

    shared = _prep_shared(block_w, block_b, fc1_w, fc1_b, fc2_w, fc2_b, gammas)
    xs = shard_x(x)
    in_maps = [dict(shared, xT=xs[r]) for r in range(N_CORES)]
    res = run_bass_kernel_spmd(nc, in_maps, list(range(N_CORES)))
    return unshard_out([res.results[r]["outT"] for r in range(N_CORES)])
